# revision 2
# baseline (speedup 1.0000x reference)
"""BN-LSTM v6: lean pipeline + low-latency SBUF->SBUF remote-DMA stats exchange.

Differences vs v2 (the previous shipped kernel):
- wi0 = x @ w_ih0 computed ON THE FLY one step ahead (no phase0, no DRAM
  round trip, no bulk stats AllGathers).
- Stats exchanged as RAW (sum, sumsq) pairs (A-exchange: 16 channels x 2
  floats; derived from bn_stats records with 4 cheap ops), aggregated with a
  3-op log-tree add instead of 16 bn_aggr. B-exchange keeps 6-float bn_stats
  records (2 bn_aggr on receive).
- exchange="rdma": each core XOR-broadcasts its slot-0 record to slot k of
  peer me^k (7 x remote_dma_broadcast preps, one trigger). Descriptor
  generation (~1us/prep on Pool Q7) is PREFETCHED one exchange ahead so only
  triggers sit on the critical path. Reader gating: vector.wait_ge(psem)
  gate + post-schedule rsem-threshold wait, parity-double-buffered gather
  tensors. All Pool DMA instructions are chained in emission order so the
  SWDGE ring order matches trigger order.
- exchange="ncfw": same record layout via ncfw AllGather (safe fallback).
- All elementwise on DVE/Scalar; Pool engine reserved for preps/triggers.
"""

import time
from contextlib import ExitStack

import numpy as np

import concourse.bass as bass
import concourse.mybir as mybir
import concourse.bacc as bacc
import concourse.tile as tile
from concourse.bass_utils import run_bass_kernel_spmd

B, T, I, H, O = 2048, 152, 75, 128, 256
NCORES = 8
BL = B // NCORES
G = 4
EPS = 1e-5

fp32 = mybir.dt.float32
AF = mybir.ActivationFunctionType
ALU = mybir.AluOpType
RG = [list(range(NCORES))]

# A-exchange record: 16 channels, layout [wh0(4) | wh1(4) | wi1(4) | wi0(4)]
CA = 16
AW = 2 * CA          # 32 floats: [a16 | b16]  (a = sum/128, b = sumsq)
BW = 12              # B-exchange: c0 bn_stats record (6) | c1 record (6)


def _build_v6(local_stats=False, repeats=1, exchange="rdma"):
    nc = bacc.Bacc("TRN2", target_bir_lowering=False, debug=False,
                   num_devices=NCORES, dynamic_dma_scratch_size=32768)

    xT = nc.dram_tensor("xT", [I, T, BL], fp32, kind="ExternalInput").ap()
    wih0 = nc.dram_tensor("wih0", [I, G * H], fp32, kind="ExternalInput").ap()
    whh0 = nc.dram_tensor("whh0", [H, G * H], fp32, kind="ExternalInput").ap()
    wih1 = nc.dram_tensor("wih1", [H, G * H], fp32, kind="ExternalInput").ap()
    whh1 = nc.dram_tensor("whh1", [H, G * H], fp32, kind="ExternalInput").ap()
    fcwT = nc.dram_tensor("fcwT", [H, O], fp32, kind="ExternalInput").ap()
    pin = {}
    for nm, w in (("gcat16", CA), ("bcat16", CA), ("ratio0", G),
                  ("ratio1", G), ("gc2", 2), ("bc2", 2)):
        pin[nm] = nc.dram_tensor(nm, [H, w], fp32, kind="ExternalInput").ap()
    y = nc.dram_tensor("y", [BL, O], fp32, kind="ExternalOutput").ap()

    rdma = (exchange == "rdma") and not local_stats
    ncfw = (exchange == "ncfw") and not local_stats

    ctx0 = ExitStack()
    deferred_waits = []
    if rdma:
        # untracked gather tensors, double-buffered by exchange parity
        gA = [ctx0.enter_context(nc.sbuf_tensor(f"gA{b_}", [H, NCORES, AW],
                                                fp32)) for b_ in range(2)]
        gB = [ctx0.enter_context(nc.sbuf_tensor(f"gB{b_}", [H, NCORES, BW],
                                                fp32)) for b_ in range(2)]
        rsemA = [nc.alloc_semaphore(f"rsemA{b_}") for b_ in range(2)]
        rsemB = [nc.alloc_semaphore(f"rsemB{b_}") for b_ in range(2)]
        lsems = [nc.alloc_semaphore(f"lsem{i}") for i in range(4)]
        psem = nc.alloc_semaphore("psem")
    ntrig = [0]
    pool_last = [None]  # emission-order chain over Pool preps+triggers

    with tile.TileContext(nc) as tc, ExitStack() as ctx:
        sb = ctx.enter_context(tc.tile_pool(name="sb", bufs=1))
        loop = ctx.enter_context(tc.tile_pool(name="loop", bufs=2))
        psum = ctx.enter_context(tc.tile_pool(name="psum", bufs=1,
                                              space="PSUM"))
        dram = ctx.enter_context(tc.tile_pool(name="dram", bufs=2,
                                              space="DRAM"))

        def load(ap_in, shape, name):
            t_ = sb.tile(shape, fp32, name=name)
            nc.sync.dma_start(t_[:], ap_in[:])
            return t_

        wih0_sb = load(wih0, [I, G * H], "wih0_sb")
        whh0_sb = load(whh0, [H, G * H], "whh0_sb")
        wih1_sb = load(wih1, [H, G * H], "wih1_sb")
        whh1_sb = load(whh1, [H, G * H], "whh1_sb")
        fcwT_sb = load(fcwT, [H, O], "fcwT_sb")
        P = {nm: load(pin[nm], [H, {"gcat16": CA, "bcat16": CA, "gc2": 2,
                                    "bc2": 2}.get(nm, G)], nm + "_sb")
             for nm in pin}

        eps_t = sb.tile([H, 1], fp32, name="eps_t")
        nc.vector.memset(eps_t[:], EPS)
        c128 = sb.tile([H, 1], fp32, name="c128")
        nc.vector.memset(c128[:], 128.0)
        cm256 = sb.tile([H, 1], fp32, name="cm256")
        nc.vector.memset(cm256[:], -1.0 / 256.0)

        # per-step A-record staging: bn_stats records for the 16 channels
        stA = sb.tile([H, CA, 6], fp32, name="stA")
        # views of the even/odd mean and M2 fields: (H, CA, 2)
        stA_m = stA[:].rearrange("p c (a b) -> p c a b", b=3)[:, :, :, 1]
        stA_M2 = stA[:].rearrange("p c (a b) -> p c a b", b=3)[:, :, :, 2]

        if local_stats:
            # slots 1-7 stay zero; tree-add over zeros == local-only stats
            gA = [sb.tile([H, NCORES, AW], fp32, name=f"gAl{b_}")
                  for b_ in range(2)]
            gB = [sb.tile([H, NCORES, BW], fp32, name=f"gBl{b_}")
                  for b_ in range(2)]
            for b_ in range(2):
                nc.vector.memset(gA[b_][:], 0.0)
                nc.vector.memset(gB[b_][:], 0.0)

        def chain(inst):
            if pool_last[0] is not None:
                bass._add_dep_helper(inst.ins, pool_last[0].ins, sync=False,
                                     reason="pool order")
            pool_last[0] = inst
            return inst

        lidx = [0]

        def preps(g, rsem):
            """Emit the 7 XOR-broadcast desc-gen preps for gather tensor g.
            local-sems rotate so no single sem accumulates past 16 bits."""
            ls = lsems[lidx[0] % 4]
            lidx[0] += 1
            for k in range(1, NCORES):
                rdests = [None] * NCORES
                rdests[k] = (0, k)
                chain(nc.gpsimd.remote_dma_broadcast(
                    g[:, k, :], g[:, 0, :], rsem, ls, rdests=rdests))

        def exchange_rd(rsem, epoch, last_writer):
            """Fire the oldest 7 prepped sends; gated on slot-0 data.
            Returns the reader-gate token."""
            trig = chain(nc.gpsimd.trigger_dma(count=NCORES - 1))
            if last_writer is not None:
                bass._add_dep_helper(trig.ins, last_writer.ins, sync=True,
                                     reason="trigger after stats write")
            trig.then_inc(psem, 1)
            ntrig[0] += 1
            gate = nc.vector.wait_ge(psem, ntrig[0])
            if last_writer is not None:
                bass._add_dep_helper(gate.ins, last_writer.ins, sync=False,
                                     reason="gate after stats")
            deferred_waits.append((gate, rsem, 14 * (epoch // 2 + 1)))
            return gate

        def gate_reader(reader, token):
            if token is not None:
                bass._add_dep_helper(reader.ins, token.ins, sync=False,
                                     reason="reader after gate")

        def exchange_ncfw(slot0_w, rec_w, tag):
            """slot0_w: (H, rec_w) SBUF AP; returns (H, NCORES, rec_w) tile."""
            bin_ = dram.tile([H, rec_w], fp32, tag=f"bin{tag}", name="bin")
            bout = dram.tile([NCORES * H, rec_w], fp32, tag=f"bout{tag}",
                             addr_space="Shared", name="bout")
            nc.sync.dma_start(bin_[:], slot0_w)
            nc.gpsimd.collective_compute(
                "AllGather", ALU.bypass, replica_groups=RG,
                ins=[bin_[:]], outs=[bout[:]])
            g8 = loop.tile([H, NCORES, rec_w], fp32, tag=f"g8{tag}",
                           name="g8")
            nc.sync.dma_start(
                g8[:], bout[:].rearrange("(r p) s -> p r s", r=NCORES))
            return g8

        # ---- step building blocks ----
        def group_stats(src_sb, ch_base):
            """Per-gate bn_stats of an SBUF (H, G, BL) tile into stA."""
            last = None
            for q in range(G):
                last = nc.vector.bn_stats(stA[:, ch_base + q, :],
                                          src_sb[:, q, :])
            return last

        def make_wi0(t_next):
            """wi0[t_next] into a fresh sbuf tile + stats into stA ch12-15."""
            xt = loop.tile([I, BL], fp32, tag="xt", bufs=3, name="xt")
            nc.sync.dma_start(xt[:], xT[:, t_next, :])
            pwx = psum.tile([H, G, BL], fp32, tag="px", bufs=1, name="pwx")
            for q in range(G):
                nc.tensor.matmul(pwx[:, q, :], wih0_sb[:, q * H:(q + 1) * H],
                                 xt[:], start=True, stop=True)
            w0 = loop.tile([H, G, BL], fp32, tag="wi0", bufs=2, name="wi0")
            nc.scalar.copy(w0[:], pwx[:])
            group_stats(w0, 12)
            return w0

        def convert_A(dst_slot0):
            """stA records -> [a16 | b16] sums into dst_slot0 (H, AW) AP.
            a = m_even + m_odd  (= sum/128)
            b = (M2_e + 128 m_e^2) + (M2_o + 128 m_o^2)  (= sumsq)"""
            nc.vector.tensor_tensor(dst_slot0[:, 0:CA], stA[:, :, 1],
                                    stA[:, :, 4], op=ALU.add)
            t1 = loop.tile([H, CA, 2], fp32, tag="cvt1", name="t1")
            nc.vector.tensor_tensor(t1[:], stA_m, stA_m, op=ALU.mult)
            t2 = loop.tile([H, CA, 2], fp32, tag="cvt2", name="t2")
            nc.vector.scalar_tensor_tensor(
                t2[:], in0=t1[:], scalar=c128[:], in1=stA_M2,
                op0=ALU.mult, op1=ALU.add)
            last = nc.vector.tensor_tensor(dst_slot0[:, CA:AW], t2[:, :, 0],
                                           t2[:, :, 1], op=ALU.add)
            return last

        def tree_fold_A(g, tokA):
            """gather (H, 8, AW) -> per-step scales."""
            r1 = loop.tile([H, 4, AW], fp32, tag="tr1", name="r1")
            a_ = nc.vector.tensor_tensor(r1[:], g[:, 0:4, :], g[:, 4:8, :],
                                         op=ALU.add)
            gate_reader(a_, tokA)
            r2 = loop.tile([H, 2, AW], fp32, tag="tr2", name="r2")
            nc.vector.tensor_tensor(r2[:], r1[:, 0:2, :], r1[:, 2:4, :],
                                    op=ALU.add)
            r3 = loop.tile([H, AW], fp32, tag="tr3", name="r3")
            nc.vector.tensor_tensor(r3[:], r2[:, 0, :], r2[:, 1, :],
                                    op=ALU.add)
            a16 = r3[:, 0:CA]
            b16 = r3[:, CA:AW]
            m16 = loop.tile([H, CA], fp32, tag="m16", name="m16")
            nc.scalar.mul(m16[:], a16, 1.0 / 16.0)
            tsq = loop.tile([H, CA], fp32, tag="tsq", name="tsq")
            nc.vector.tensor_tensor(tsq[:], a16, a16, op=ALU.mult)
            v1 = loop.tile([H, CA], fp32, tag="v1f", name="v1")
            nc.scalar.mul(v1[:], b16, 1.0 / 2048.0)
            var = loop.tile([H, CA], fp32, tag="varf", name="var")
            nc.vector.scalar_tensor_tensor(
                var[:], in0=tsq[:], scalar=cm256[:], in1=v1[:],
                op0=ALU.mult, op1=ALU.add)
            sd = loop.tile([H, CA], fp32, tag="sdf", name="sd")
            nc.scalar.activation(sd[:], var[:], AF.Sqrt, bias=eps_t[:])
            r16 = loop.tile([H, CA], fp32, tag="r16", name="r16")
            nc.vector.reciprocal(r16[:], sd[:])
            S16 = loop.tile([H, CA], fp32, tag="S16", name="S16")
            nc.vector.tensor_tensor(S16[:], r16[:], P["gcat16"][:],
                                    op=ALU.mult)
            TM = loop.tile([H, CA], fp32, tag="TM16", name="TM")
            nc.vector.tensor_tensor(TM[:], m16[:], S16[:], op=ALU.mult)
            SH = loop.tile([H, CA], fp32, tag="SH16", name="SH")
            nc.vector.tensor_tensor(SH[:], P["bcat16"][:], TM[:],
                                    op=ALU.subtract)
            u0 = loop.tile([H, G], fp32, tag="u0", name="u0")
            nc.vector.tensor_tensor(u0[:], P["ratio0"][:], sd[:, 0:4],
                                    op=ALU.mult)
            nc.vector.tensor_tensor(u0[:], u0[:], r16[:, 12:16], op=ALU.mult)
            u1 = loop.tile([H, G], fp32, tag="u1", name="u1")
            nc.vector.tensor_tensor(u1[:], P["ratio1"][:], sd[:, 4:8],
                                    op=ALU.mult)
            nc.vector.tensor_tensor(u1[:], u1[:], r16[:, 8:12], op=ALU.mult)
            v0 = loop.tile([H, G], fp32, tag="v0", name="v0")
            nc.vector.tensor_tensor(v0[:], SH[:, 0:4], SH[:, 12:16],
                                    op=ALU.add)
            v1g = loop.tile([H, G], fp32, tag="v1g", name="v1g")
            nc.vector.tensor_tensor(v1g[:], SH[:, 4:8], SH[:, 8:12],
                                    op=ALU.add)
            return dict(S=S16, u0=u0, u1=u1, v0=v0, v1=v1g)

        def gates_layer(wh_sb, wi, u, vv, S16, soff, tagp):
            """All-SBUF gate computation: gin = wh + u*wi, ga = act(S*gin+v)."""
            gin = loop.tile([H, G, BL], fp32, tag="gin" + tagp, bufs=2,
                            name="gin" + tagp)
            for q in range(G):
                nc.vector.scalar_tensor_tensor(
                    gin[:, q, :], in0=wi[:, q, :], scalar=u[:, q:q + 1],
                    in1=wh_sb[:, q, :], op0=ALU.mult, op1=ALU.add)
            ga = loop.tile([H, G, BL], fp32, tag="ga" + tagp, bufs=2,
                           name="ga" + tagp)
            for q, fn in ((0, AF.Sigmoid), (1, AF.Sigmoid), (3, AF.Tanh),
                          (2, AF.Sigmoid)):
                nc.scalar.activation(ga[:, q, :], gin[:, q, :], fn,
                                     bias=vv[:, q:q + 1],
                                     scale=S16[:, soff + q:soff + q + 1])
            return ga

        def cell_update(ga, c_old, tagc):
            t1 = loop.tile([H, BL], fp32, tag="ct1" + tagc, name="t1c")
            nc.vector.tensor_tensor(t1[:], ga[:, 1, :], ga[:, 3, :],
                                    op=ALU.mult)
            t2 = loop.tile([H, BL], fp32, tag="ct2" + tagc, name="t2c")
            nc.vector.tensor_tensor(t2[:], ga[:, 0, :], c_old[:],
                                    op=ALU.mult)
            cn = loop.tile([H, BL], fp32, tag="c" + tagc, name="cn")
            nc.vector.tensor_tensor(cn[:], t1[:], t2[:], op=ALU.add)
            return cn

        # ================= pipeline =================
        ecnt = {"a": 0, "b": 0}
        h_fin = None

        for _rep in range(repeats):
            h0 = loop.tile([H, BL], fp32, tag="h0", name="h0")
            c0 = loop.tile([H, BL], fp32, tag="c0s", name="c0")
            h1 = loop.tile([H, BL], fp32, tag="h1", name="h1")
            c1 = loop.tile([H, BL], fp32, tag="c1s", name="c1")
            for t_ in (h0, c0, h1, c1):
                nc.vector.memset(t_[:], 0.0)

            # -------- prologue: assemble A@0 --------
            nc.vector.memset(stA[:], 0.0)
            wi_cur = make_wi0(0)
            pw0 = psum.tile([H, G, BL], fp32, tag="g0", bufs=1, name="pw0")
            for q in range(G):
                nc.tensor.matmul(pw0[:, q, :], whh0_sb[:, q * H:(q + 1) * H],
                                 h0[:], start=True, stop=True)
            wh0sb = loop.tile([H, G, BL], fp32, tag="s0", bufs=2, name="s0")
            nc.scalar.copy(wh0sb[:], pw0[:])
            group_stats(wh0sb, 0)
            # wh1 for step 1 (L1's timestep 0): whh1 @ h1_init(=0)
            pw1 = psum.tile([H, G, BL], fp32, tag="g1", bufs=1, name="pw1")
            for q in range(G):
                nc.tensor.matmul(pw1[:, q, :], whh1_sb[:, q * H:(q + 1) * H],
                                 h1[:], start=True, stop=True)
            wh1sb = loop.tile([H, G, BL], fp32, tag="s1", bufs=2, name="s1")
            nc.scalar.copy(wh1sb[:], pw1[:])
            wi1sb = None

            ea = ecnt["a"]
            ecnt["a"] += 1
            if rdma:
                zb = nc.vector.memset(gB[ecnt["b"] % 2][:, 0, 6:12], 0.0)
                convA = convert_A(gA[ea % 2][:, 0, :])
                bass._add_dep_helper(convA.ins, zb.ins, sync=False,
                                     reason="order")
                preps(gA[ea % 2], rsemA[ea % 2])
                preps(gB[ecnt["b"] % 2], rsemB[ecnt["b"] % 2])
                tokA = exchange_rd(rsemA[ea % 2], ea, convA)
                gA_t = gA[ea % 2]
            else:
                slotA = loop.tile([H, AW], fp32, tag="slotA", name="slotA")
                convA = convert_A(slotA[:])
                if ncfw:
                    gA_t = exchange_ncfw(slotA[:], AW, "A")
                else:
                    gA_t = gA[ea % 2]
                    nc.scalar.copy(gA_t[:, 0, :], slotA[:])
                    nc.vector.memset(gB[ecnt["b"] % 2][:, 0, 6:12], 0.0)
                tokA = None

            for t in range(T + 1):
                has0 = t < T
                has1 = t >= 1
                # ---- A@t arrival + fold ----
                sc = tree_fold_A(gA_t, tokA)
                # ---- prefetch next wi0 (t+1) ----
                wi_next = None
                if t + 1 < T:
                    wi_next = make_wi0(t + 1)
                # ---- gates + cell updates + B-record ----
                eb = ecnt["b"]
                ecnt["b"] += 1
                if rdma or local_stats:
                    gB_t = gB[eb % 2]
                    slotB0 = gB_t[:, 0, :]
                else:
                    slotB = loop.tile([H, BW], fp32, tag="slotB",
                                      name="slotB")
                    slotB0 = slotB[:]
                ga0 = ga1 = None
                c0n = c1n = None
                if has0:
                    ga0 = gates_layer(wh0sb, wi_cur, sc["u0"], sc["v0"],
                                      sc["S"], 0, "0")
                    c0n = cell_update(ga0, c0, "0")
                    lastB = nc.vector.bn_stats(slotB0[:, 0:6], c0n[:])
                else:
                    lastB = nc.vector.memset(slotB0[:, 0:6], 0.0)
                if has1:
                    ga1 = gates_layer(wh1sb, wi1sb, sc["u1"], sc["v1"],
                                      sc["S"], 4, "1")
                    c1n = cell_update(ga1, c1, "1")
                    lb = nc.vector.bn_stats(slotB0[:, 6:12], c1n[:])
                    bass._add_dep_helper(lb.ins, lastB.ins, sync=False,
                                         reason="order B writes")
                    lastB = lb
                # ---- B exchange ----
                if rdma:
                    tokB = exchange_rd(rsemB[eb % 2], eb, lastB)
                elif ncfw:
                    gB_t = exchange_ncfw(slotB0, BW, "B")
                    tokB = None
                else:
                    nc.scalar.copy(gB_t[:, 0, :], slotB0)
                    tokB = None
                # ---- prefetch preps for A@t+1 / B@t+1 ----
                if rdma and t < T:
                    preps(gA[ecnt["a"] % 2], rsemA[ecnt["a"] % 2])
                    preps(gB[ecnt["b"] % 2], rsemB[ecnt["b"] % 2])
                # ---- B fold ----
                mvc2 = loop.tile([H, 2, 2], fp32, tag="mvc2", name="mvc2")
                for k in range(2):
                    ag = nc.vector.bn_aggr(mvc2[:, k, :],
                                           gB_t[:, :, 6 * k:6 * (k + 1)])
                    gate_reader(ag, tokB)
                sdc2 = loop.tile([H, 2], fp32, tag="sdc2", name="sdc2")
                nc.scalar.activation(sdc2[:], mvc2[:, :, 1], AF.Sqrt,
                                     bias=eps_t[:])
                rc2 = loop.tile([H, 2], fp32, tag="rc2", name="rc2")
                nc.vector.reciprocal(rc2[:], sdc2[:])
                scc = loop.tile([H, 2], fp32, tag="scc", name="scc")
                nc.vector.tensor_tensor(scc[:], rc2[:], P["gc2"][:],
                                        op=ALU.mult)
                tmc = loop.tile([H, 2], fp32, tag="tmc", name="tmc")
                nc.vector.tensor_tensor(tmc[:], mvc2[:, :, 0], scc[:],
                                        op=ALU.mult)
                shc = loop.tile([H, 2], fp32, tag="shc", name="shc")
                nc.vector.tensor_tensor(shc[:], P["bc2"][:], tmc[:],
                                        op=ALU.subtract)
                # ---- h updates ----
                h0n = h1n = None
                if has0:
                    tn0 = loop.tile([H, BL], fp32, tag="tn0", name="tn0")
                    nc.scalar.activation(tn0[:], c0n[:], AF.Tanh,
                                         bias=shc[:, 0:1], scale=scc[:, 0:1])
                    h0n = loop.tile([H, BL], fp32, tag="h0", name="h0n")
                    nc.vector.tensor_tensor(h0n[:], ga0[:, 2, :], tn0[:],
                                            op=ALU.mult)
                    h0 = h0n
                    c0 = c0n
                if has1:
                    tn1 = loop.tile([H, BL], fp32, tag="tn1", name="tn1")
                    nc.scalar.activation(tn1[:], c1n[:], AF.Tanh,
                                         bias=shc[:, 1:2], scale=scc[:, 1:2])
                    h1n = loop.tile([H, BL], fp32, tag="h1", name="h1n")
                    nc.vector.tensor_tensor(h1n[:], ga1[:, 2, :], tn1[:],
                                            op=ALU.mult)
                    h1 = h1n
                    c1 = c1n
                if t == T:
                    h_fin = h1
                    break
                # ---- assemble A@t+1: wh0[t+1], wi1[t], wh1[t] ----
                pw0 = psum.tile([H, G, BL], fp32, tag="g0", bufs=1,
                                name="pw0n")
                for q in range(G):
                    nc.tensor.matmul(pw0[:, q, :],
                                     whh0_sb[:, q * H:(q + 1) * H],
                                     h0n[:], start=True, stop=True)
                wh0sb = loop.tile([H, G, BL], fp32, tag="s0", bufs=2,
                                  name="s0n")
                nc.scalar.copy(wh0sb[:], pw0[:])
                group_stats(wh0sb, 0)
                pwm = psum.tile([H, G, BL], fp32, tag="w1", bufs=1,
                                name="pwm")
                for q in range(G):
                    nc.tensor.matmul(pwm[:, q, :],
                                     wih1_sb[:, q * H:(q + 1) * H],
                                     h0n[:], start=True, stop=True)
                wi1n = loop.tile([H, G, BL], fp32, tag="wi1sb", bufs=2,
                                 name="wi1n")
                nc.scalar.copy(wi1n[:], pwm[:])
                group_stats(wi1n, 8)
                wi1sb = wi1n
                pw1 = psum.tile([H, G, BL], fp32, tag="g1", bufs=1,
                                name="pw1n")
                h1src = h1n if has1 else h1
                for q in range(G):
                    nc.tensor.matmul(pw1[:, q, :],
                                     whh1_sb[:, q * H:(q + 1) * H],
                                     h1src[:], start=True, stop=True)
                wh1sb = loop.tile([H, G, BL], fp32, tag="s1", bufs=2,
                                  name="s1n")
                nc.scalar.copy(wh1sb[:], pw1[:])
                lastA = group_stats(wh1sb, 4)
                if t + 1 >= T:
                    # no wi0[t+1] / wh0 is bogus-but-unused; zero wi0 chans
                    lastA = nc.vector.memset(stA[:, 12:16, :], 0.0)
                # ---- trigger A@t+1 ----
                ea = ecnt["a"]
                ecnt["a"] += 1
                if rdma:
                    convA = convert_A(gA[ea % 2][:, 0, :])
                    tokA = exchange_rd(rsemA[ea % 2], ea, convA)
                    gA_t = gA[ea % 2]
                else:
                    slotA = loop.tile([H, AW], fp32, tag="slotA",
                                      name="slotA")
                    convA = convert_A(slotA[:])
                    if ncfw:
                        gA_t = exchange_ncfw(slotA[:], AW, "A")
                    else:
                        gA_t = gA[ea % 2]
                        nc.scalar.copy(gA_t[:, 0, :], slotA[:])
                    tokA = None
                wi_cur = wi_next

        # ---------------- final FC ----------------
        yo = None
        for ci in range(2):
            pf = psum.tile([H, O], fp32, tag="w1", name="pf")
            nc.tensor.matmul(pf[:], h_fin[:, ci * H:(ci + 1) * H], fcwT_sb[:],
                             start=True, stop=True)
            yo = loop.tile([H, O], fp32, tag="yo", name="yo")
            nc.scalar.copy(yo[:], pf[:])
            nc.sync.dma_start(
                y[:].rearrange("(c p) o -> c p o", c=2)[ci], yo[:])

        if rdma:
            # End-of-dispatch barrier + semaphore clear: a re-dispatched NEFF
            # must start with zeroed exchange semaphores, and no core may
            # clear before every core has consumed all remote arrivals.
            bbin = dram.tile([H, 1], fp32, tag="barin", name="barin")
            bbout = dram.tile([NCORES * H, 1], fp32, tag="barout",
                              addr_space="Shared", name="barout")
            nc.sync.dma_start(bbin[:], yo[:, 0:1])
            bar = nc.gpsimd.collective_compute(
                "AllGather", ALU.bypass, replica_groups=RG,
                ins=[bbin[:]], outs=[bbout[:]])
            for sem in (rsemA[0], rsemA[1], rsemB[0], rsemB[1], psem,
                        *lsems):
                cl = nc.vector.sem_clear(sem)
                bass._add_dep_helper(cl.ins, bar.ins, sync=True,
                                     reason="clear after barrier")

    for binst, sem, thresh in deferred_waits:
        binst.wait_op(sem, thresh, "sem-ge")
    nc.compile()
    ctx0.close()
    return nc


def _prep_inputs_v6(sequences, w_ih0, w_hh0, b0, g_ih0, be_ih0, g_hh0, be_hh0,
                    g_c0, be_c0, w_ih1, w_hh1, b1, g_ih1, be_ih1, g_hh1,
                    be_hh1, g_c1, be_c1, fc_w, fc_b):
    f32 = np.float32

    def pg(v):  # (512,) -> (128, 4)
        return np.ascontiguousarray(np.asarray(v, f32).reshape(G, H).T)

    common = {
        "wih0": np.ascontiguousarray(np.asarray(w_ih0, f32)),
        "whh0": np.ascontiguousarray(np.asarray(w_hh0, f32)),
        "wih1": np.ascontiguousarray(np.asarray(w_ih1, f32)),
        "whh1": np.ascontiguousarray(np.asarray(w_hh1, f32)),
        "fcwT": np.ascontiguousarray(np.asarray(fc_w, f32).T),
        # channel layout: [wh0 | wh1 | wi1 | wi0]
        "gcat16": np.concatenate([pg(g_hh0), pg(g_hh1), pg(g_ih1),
                                  pg(g_ih0)], axis=1),
        "bcat16": np.concatenate(
            [pg(be_hh0), pg(be_hh1),
             pg(np.asarray(be_ih1) + np.asarray(b1)),
             pg(np.asarray(be_ih0) + np.asarray(b0))], axis=1),
        "ratio0": pg(np.asarray(g_ih0) / np.asarray(g_hh0)),
        "ratio1": pg(np.asarray(g_ih1) / np.asarray(g_hh1)),
        "gc2": np.stack([np.asarray(g_c0, f32),
                         np.asarray(g_c1, f32)], axis=1).copy(),
        "bc2": np.stack([np.asarray(be_c0, f32),
                         np.asarray(be_c1, f32)], axis=1).copy(),
    }
    seq = np.asarray(sequences, f32)
    in_maps = []
    for c in range(NCORES):
        m = dict(common)
        m["xT"] = np.ascontiguousarray(
            seq[c * BL:(c + 1) * BL].transpose(2, 1, 0))
        in_maps.append(m)
    return in_maps


# ---------------------------------------------------------------------------
# harness entry points
# ---------------------------------------------------------------------------
EXCHANGE = "ncfw"   # "ncfw" (proven) | "rdma" (slower on this fabric)

_NC_CACHE = None


def _get_nc():
    global _NC_CACHE
    if _NC_CACHE is None:
        _NC_CACHE = _build_v6(exchange=EXCHANGE)
    return _NC_CACHE


def build_for_timing(repeats=1):
    return _build_v6(exchange=EXCHANGE, repeats=repeats)


def _prep_inputs(**inputs):
    return _prep_inputs_v6(**inputs)


def kernel(**inputs):
    nc = _get_nc()
    in_maps = _prep_inputs_v6(**inputs)
    last_exc = None
    for attempt in range(3):
        try:
            res = run_bass_kernel_spmd(nc, in_maps,
                                       core_ids=list(range(NCORES)),
                                       trace=False)
            break
        except Exception as e:  # transient runtime INTERNAL errors observed
            last_exc = e
            time.sleep(5.0 * (attempt + 1))
    else:
        raise last_exc
    ys = [res.results[c]["y"] for c in range(NCORES)]
    out = np.concatenate(ys, axis=0)  # (B, O)
    out = out + np.asarray(inputs["fc_b"], np.float32)[None, :]
    return out.astype(np.float32)
